# revision 15
# baseline (speedup 1.0000x reference)
"""DharmaAttention TRN2 kernel — fused single-pass version.

Full-input contract: kernel(**inputs) takes the unsharded inputs and returns
the full [2, 2048, 2048] output.

Sharding (8 cores): 2-way data-parallel over batch x 4-way tensor-parallel
over head groups (4 heads of head_dim 128 per core). Wq/Wk/Wv are split
column-wise (output channels) per head group, Wo row-wise; each core produces
a partial output projection for its batch element and the host sums the 4
partials per batch.

Single fused pass per core: for each 512-column chunk of the sequence,
project Q/K for the 4 heads (+RoPE), project V, run causal attention for
that q-chunk against all K/V accumulated so far (all resident in SBUF),
and immediately run the Wo projection for the chunk. x is read once; q/k/v
never round-trip through DRAM. All matmul operands are bf16 (full PE rate,
half the DMA/LDWEIGHTS cost of f32); accumulation stays fp32 in PSUM.

Engine assignment: PE matmuls; RoPE(Q) + mask + softmax-normalize on Vector;
RoPE(K) + output staging copies on GpSimd; exp + V staging on Scalar/ACT.
Softmax skips the max subtraction: scores are O(+-6), exp is safe in fp32,
and softmax is shift-invariant so the result matches the reference.
"""

import math
import sys

sys.path.insert(0, "/opt/trn_rl_repo")

import numpy as np

B = 2
S = 2048
H = 2048
NH = 16
HD = 128
THETA = 10000.0
G = 4  # heads per core (tensor-parallel group size NH / 4)
GC = G * HD  # channels per core = 512
NHT = H // 128  # 16 contraction tiles
SC = 512  # seq chunk
NSC = S // SC  # 4
NKB = S // 128  # 16 k blocks
INV_SQRT_HD = 1.0 / math.sqrt(HD)

_prog_cache = {}

# test-harness hooks (the grading path leaves these at defaults)
TRACE = False
LAST_RESULTS = None


def _split_multi_waits(nc):
    """The walrus build here accepts at most ONE sync wait per instruction
    ('Too many sync wait commands'). Hoist extra on_wait entries into no-op
    instructions inserted just before, on the same engine."""
    import concourse.mybir as mybir

    for f in nc.m.functions:
        for b in f.blocks:
            out = []
            changed = False
            for inst in b.instructions:
                si = getattr(inst, "sync_info", None)
                waits = list(si.on_wait) if si is not None and si.on_wait else []
                if len(waits) > 1:
                    for k, w in enumerate(waits[:-1]):
                        nop = mybir.InstNoOp(
                            name=f"{inst.name}-w{k}",
                            sync_info=mybir.SyncInfo(on_wait=[w], on_update=[]),
                        )
                        nop.engine = inst.engine
                        out.append(nop)
                    inst.sync_info = mybir.SyncInfo(
                        on_wait=[waits[-1]], on_update=list(si.on_update or [])
                    )
                    changed = True
                out.append(inst)
            if changed:
                b.instructions = out


def _build_nc():
    import concourse.bass as bass
    import concourse.mybir as mybir
    import concourse.tile as tile

    F32 = mybir.dt.float32
    BF16 = mybir.dt.bfloat16
    MULT = mybir.AluOpType.mult
    ADD = mybir.AluOpType.add
    EXP = mybir.ActivationFunctionType.Exp

    nc = bass.Bass("TRN2", target_bir_lowering=False, debug=False)

    xT = nc.dram_tensor("xT", [H, S], BF16, kind="ExternalInput").ap()
    wqT = nc.dram_tensor("wqT", [H, GC], BF16, kind="ExternalInput").ap()
    wkT = nc.dram_tensor("wkT", [H, GC], BF16, kind="ExternalInput").ap()
    wvT = nc.dram_tensor("wvT", [H, GC], BF16, kind="ExternalInput").ap()
    woc = nc.dram_tensor("woc", [GC, H], BF16, kind="ExternalInput").ap()
    cosT_d = nc.dram_tensor("cosT", [HD, S], BF16, kind="ExternalInput").ap()
    sinN_d = nc.dram_tensor("sinN", [HD // 2, S], BF16, kind="ExternalInput").ap()
    maskd_d = nc.dram_tensor("maskd", [128, 4, SC], BF16, kind="ExternalInput").ap()
    yT = nc.dram_tensor("yT", [H, S], F32, kind="ExternalOutput").ap()

    with tile.TileContext(nc) as tc:
        with (
            tc.tile_pool(name="consts", bufs=1) as consts,
            tc.tile_pool(name="xpool", bufs=2) as xpool,
            tc.tile_pool(name="qpool", bufs=2) as qpool,
            tc.tile_pool(name="rpool", bufs=1) as rpool,
            tc.tile_pool(name="prpool", bufs=2) as prpool,
            tc.tile_pool(name="bcpool", bufs=1) as bcpool,
            tc.tile_pool(name="opool", bufs=2) as opool,
            tc.tile_pool(name="ypool", bufs=2) as ypool,
            tc.tile_pool(name="ps", bufs=1, space="PSUM") as ps,
        ):
            # weights needed first come first so their DMAs land early:
            # the first matmul needs wq + x chunk 0, so those two lead.
            wq_sb = consts.tile([128, NHT, GC], BF16)
            wk_sb = consts.tile([128, NHT, GC], BF16)
            nc.sync.dma_start(out=wq_sb, in_=wqT.rearrange("(t p) o -> p t o", p=128))
            x0_sb = xpool.tile([128, NHT, SC], BF16)
            nc.sync.dma_start(
                out=x0_sb, in_=xT[:, 0:SC].rearrange("(t p) s -> p t s", p=128)
            )
            nc.sync.dma_start(out=wk_sb, in_=wkT.rearrange("(t p) o -> p t o", p=128))
            cosT = consts.tile([HD, S], BF16)
            sinN = consts.tile([HD // 2, S], BF16)  # rows hold -sin only
            nc.sync.dma_start(out=cosT, in_=cosT_d)
            nc.sync.dma_start(out=sinN, in_=sinN_d)
            maskd = consts.tile([128, 4, SC], BF16)
            nc.sync.dma_start(out=maskd, in_=maskd_d)
            wv_sb = consts.tile([128, NHT, GC], BF16)
            nc.sync.dma_start(out=wv_sb, in_=wvT.rearrange("(t p) o -> p t o", p=128))
            woc_sb = consts.tile([128, G, H], BF16)
            nc.sync.dma_start(out=woc_sb, in_=woc.rearrange("(c p) o -> p c o", p=128))

            ones_f = consts.tile([128, 128], F32)
            ones_mat = consts.tile([128, 128], BF16)
            nc.vector.memset(ones_f, 1.0)
            nc.vector.tensor_copy(ones_mat, ones_f)

            # persistent K (roped) and V for the whole sequence
            k_sb = consts.tile([128, G, S], BF16)
            v_sb = consts.tile([128, NKB, GC], BF16)

            for sc in range(NSC):
                ssl = slice(sc * SC, (sc + 1) * SC)
                if sc == 0:
                    x_sb = x0_sb
                else:
                    x_sb = xpool.tile([128, NHT, SC], BF16)
                    nc.sync.dma_start(
                        out=x_sb, in_=xT[:, ssl].rearrange("(t p) s -> p t s", p=128)
                    )

                # ---- Q/K projections + RoPE ----
                q_sb = qpool.tile([128, G, SC], BF16)
                for h in range(G):
                    for which, (w_sb, eng, pool) in enumerate(
                        (
                            (wq_sb, nc.vector, rpool),
                            (wk_sb, nc.vector, rpool),
                        )
                    ):
                        pqk = ps.tile([128, SC], F32, tag="pp", bufs=2)
                        for ht in range(NHT):
                            nc.tensor.matmul(
                                pqk,
                                w_sb[:, ht, h * 128 : (h + 1) * 128],
                                x_sb[:, ht, :],
                                start=(ht == 0),
                                stop=(ht == NHT - 1),
                            )
                        # RoPE: dst = pqk * cos + rot_half(pqk) * sin
                        # sinN holds -sin; the second half negates via the
                        # fused scalar (-pqk) * (-sin) = pqk * sin.
                        tmp = pool.tile([128, SC], F32, tag="t")
                        eng.tensor_tensor(
                            out=tmp[0:64, :], in0=pqk[64:128, :],
                            in1=sinN[:, ssl], op=MULT,
                        )
                        eng.scalar_tensor_tensor(
                            out=tmp[64:128, :], in0=pqk[0:64, :],
                            scalar=-1.0, in1=sinN[:, ssl],
                            op0=MULT, op1=MULT,
                        )
                        cpart = pool.tile([128, SC], F32, tag="c")
                        eng.tensor_tensor(
                            out=cpart, in0=pqk, in1=cosT[:, ssl], op=MULT
                        )
                        dst = q_sb[:, h, :] if which == 0 else k_sb[:, h, ssl]
                        eng.tensor_tensor(out=dst, in0=cpart, in1=tmp, op=ADD)

                # ---- V projection ----
                for st2 in range(SC // 128):
                    st = sc * (SC // 128) + st2
                    pv = ps.tile([128, GC], F32, tag="pp", bufs=2)
                    for ht in range(NHT):
                        nc.tensor.matmul(
                            pv,
                            x_sb[:, ht, st2 * 128 : (st2 + 1) * 128],
                            wv_sb[:, ht, :],
                            start=(ht == 0),
                            stop=(ht == NHT - 1),
                        )
                    nc.scalar.copy(v_sb[:, st, :], pv)

                # ---- attention for q-chunk sc over k blocks 0..nk ----
                nk = 4 * (sc + 1)
                outh = opool.tile([128, G, SC], BF16)
                for h in range(G):
                    po = ps.tile([128, SC], F32, tag="po", bufs=2)
                    pbs = ps.tile([128, SC], F32, tag="pbs", bufs=2)
                    for ki in range(nk):
                        psc = ps.tile([128, SC], F32, tag="psc", bufs=2)
                        nc.tensor.matmul(
                            psc,
                            k_sb[:, h, ki * 128 : (ki + 1) * 128],
                            q_sb[:, h, :],
                            start=True,
                            stop=True,
                        )
                        m = ki - 4 * sc
                        pr = prpool.tile([128, SC], BF16, tag="pr")
                        if m >= 0:
                            prf = prpool.tile([128, SC], BF16, tag="prf")
                            nc.scalar.activation(
                                prf, psc, EXP, scale=INV_SQRT_HD
                            )
                            nc.gpsimd.tensor_tensor(
                                out=pr, in0=prf, in1=maskd[:, m, :], op=MULT
                            )
                        else:
                            nc.scalar.activation(pr, psc, EXP, scale=INV_SQRT_HD)
                        nc.tensor.matmul(
                            po,
                            v_sb[:, ki, h * 128 : (h + 1) * 128],
                            pr,
                            start=(ki == 0),
                            stop=(ki == nk - 1),
                        )
                        nc.tensor.matmul(
                            pbs, ones_mat, pr,
                            start=(ki == 0), stop=(ki == nk - 1),
                        )
                    bc = bcpool.tile([128, SC], F32, tag="bc")
                    nc.vector.reciprocal(out=bc, in_=pbs)
                    nc.vector.tensor_tensor(
                        out=outh[:, h, :], in0=po, in1=bc, op=MULT
                    )

                # ---- output projection for this chunk ----
                for ot in range(NHT):
                    py = ps.tile([128, SC], F32, tag="pp", bufs=2)
                    for h in range(G):
                        nc.tensor.matmul(
                            py,
                            woc_sb[:, h, ot * 128 : (ot + 1) * 128],
                            outh[:, h, :],
                            start=(h == 0),
                            stop=(h == G - 1),
                        )
                    ysf = ypool.tile([128, SC], F32)
                    nc.scalar.copy(ysf, py)
                    nc.sync.dma_start(out=yT[ot * 128 : (ot + 1) * 128, ssl], in_=ysf)

    _split_multi_waits(nc)
    return nc


def _host_tables():
    inv_freq = 1.0 / (THETA ** (np.arange(0, HD, 2, dtype=np.float32) / HD))
    t = np.arange(S, dtype=np.float32)
    freqs = np.einsum("i,j->ij", t, inv_freq)  # [S, 64]
    cos_h = np.cos(freqs).astype(np.float32)  # [S, 64]
    sin_h = np.sin(freqs).astype(np.float32)
    cosT = np.empty((HD, S), np.float32)
    cosT[0:64] = cos_h.T
    cosT[64:128] = cos_h.T
    sinN = np.ascontiguousarray(-sin_h.T)  # [64, S]
    p = np.arange(128)[:, None]
    s = np.arange(SC)[None, :]
    maskd = np.empty((128, 4, SC), np.float32)
    for m in range(4):
        maskd[:, m, :] = (s >= 128 * m + p).astype(np.float32)
    return cosT, sinN, maskd


def _prep_core_inputs(hidden_states, Wq, Wk, Wv, Wo, b, g):
    import ml_dtypes

    BF = ml_dtypes.bfloat16
    cosT, sinN, maskd = _host_tables()
    rows = slice(g * GC, (g + 1) * GC)
    return {
        "xT": np.ascontiguousarray(hidden_states[b].T).astype(BF),
        "wqT": np.ascontiguousarray(Wq[rows, :].T).astype(BF),
        "wkT": np.ascontiguousarray(Wk[rows, :].T).astype(BF),
        "wvT": np.ascontiguousarray(Wv[rows, :].T).astype(BF),
        "woc": np.ascontiguousarray(Wo[:, rows].T).astype(BF),
        "cosT": cosT.astype(BF),
        "sinN": sinN.astype(BF),
        "maskd": maskd.astype(BF),
    }


def kernel(hidden_states, Wq, Wk, Wv, Wo):
    from concourse import bass_utils

    hidden_states = np.asarray(hidden_states, dtype=np.float32)
    Wq = np.asarray(Wq, dtype=np.float32)
    Wk = np.asarray(Wk, dtype=np.float32)
    Wv = np.asarray(Wv, dtype=np.float32)
    Wo = np.asarray(Wo, dtype=np.float32)

    if "nc" not in _prog_cache:
        _prog_cache["nc"] = _build_nc()
    nc = _prog_cache["nc"]

    in_maps = []
    for c in range(8):
        b, g = divmod(c, 4)
        in_maps.append(_prep_core_inputs(hidden_states, Wq, Wk, Wv, Wo, b, g))

    res = bass_utils.run_bass_kernel_spmd(
        nc, in_maps, core_ids=list(range(8)), trace=TRACE
    )
    global LAST_RESULTS
    LAST_RESULTS = res

    out = np.zeros((B, S, H), np.float32)
    for c in range(8):
        b = c // 4
        out[b] += res.results[c]["yT"].T
    return out
